# revision 1
# baseline (speedup 1.0000x reference)
"""Trainium2 Bass kernel for nn_DenseGraphWaveletLayer (v4).

out[:, l, :] = phi_l @ diag(theta) @ phi_inv_l @ (features[:, l, :] @ W)

v4 strategy (8 cores SPMD, one program, per-core data):
  - spmm1 (phi_inv): HBM dma_gather (transpose=False, 256B rows) from the
    bf16 feature table, one-hot sel matmuls accumulate U^T in PSUM, then
    z = U @ W; diag folded into spmm1 edge values on host.
  - z shards AllGather'd in wrapped layout -> staged once per scale into a
    resident SBUF token table [128 part, 392 ranks x 256B].
  - spmm2 (phi): SBUF-source dma_gather (transpose=True) -> G^T [ch, e]
    tiles; an identity matmul (lhsT=G^T) transposes each tile back to
    [e, ch] in PSUM; scalar copy applies per-edge vals (per-partition
    activation scale) and casts to bf16; pure 0/1 sel matmuls accumulate
    the fp32 output rows.
  - Exact per-(l, m, block) run sizes, equalized across cores with valid
    dup-pads (idx 0 / rl 255) and descriptor-free trailing -1 pads.
  - Gather chunks up to 4096 idxs with single_packet=False.
"""

import os
import sys
import types

import numpy as np
import ml_dtypes

BF16 = ml_dtypes.bfloat16

N = 50000
L = 4
C = 128
NCORES = 8
BLK = 128
NB_TOT = (N + BLK - 1) // BLK            # 391
NBPC = (NB_TOT + NCORES - 1) // NCORES   # 49
SHARD = NBPC * BLK                       # 6272
TBL = NCORES * SHARD                     # 50176
HALF = 32768
CAP = 4096
KSEL = 8
NQUEUES = 4
MINCNT = 16


def _install_hook_stub():
    try:
        import antenv
    except ImportError:
        return
    try:
        from antenv import axon_hooks  # noqa: F401
        return
    except ImportError:
        pass
    mod = types.ModuleType("antenv.axon_hooks")
    mod._hook = None
    mod.set_axon_ntff_profile_hook = lambda h: setattr(mod, "_hook", h)
    mod.get_axon_ntff_profile_hook = lambda: mod._hook
    sys.modules["antenv.axon_hooks"] = mod
    antenv.axon_hooks = mod


def _ceil128(x):
    return -(-x // BLK) * BLK


def _tok2(cols):
    """z-table token id: core-section-major wrapped layout."""
    cblk = cols >> 7
    return (((cblk & (NCORES - 1)) * NBPC + (cblk >> 3)) << 7) | (cols & 127)


class MatPlan:
    """Per-(l) slot layout for one sparse matrix, shared by all cores."""

    def __init__(self):
        self.maxlo = None   # [L, NBPC]
        self.maxhi = None
        self.plo = None     # padded slots
        self.phi = None
        self.blkoff = None  # [L, NBPC] slot offset of block
        self.auxoff = None  # [L, NBPC] aux col offset of block
        self.nt = None      # [L, NBPC] tiles per block
        self.tot_slots = 0
        self.tot_aux = 0


def _preprocess_mat(rows_l, ckey_l, vals_l):
    """rows/ckey/vals: lists of L arrays. Returns (plan, idx [8,128,S16],
    aux [8,128,A])."""
    plan = MatPlan()
    cnt = np.zeros((L, NCORES, NBPC, 2), np.int64)
    per_l = []
    for l in range(L):
        rows, ckey, vals = rows_l[l], ckey_l[l], vals_l[l]
        core = (rows >> 7) & (NCORES - 1)
        k = rows >> 10
        rl = (rows & 127).astype(np.int16)
        hi = (ckey >= HALF).astype(np.int64)
        idxv = (ckey - HALF * hi).astype(np.int16)
        g = ((core * NBPC + k) * 2 + hi).astype(np.int64)
        cnt[l] = np.bincount(g, minlength=NCORES * NBPC * 2).reshape(
            NCORES, NBPC, 2)
        per_l.append((g, idxv, rl, vals.astype(np.float32)))

    maxc = np.maximum(cnt.max(axis=1), MINCNT)          # [L, NBPC, 2]
    plan.maxlo, plan.maxhi = maxc[..., 0], maxc[..., 1]
    plo = ((plan.maxlo + 127) // 128) * 128
    phi = ((plan.maxhi + 127) // 128) * 128
    plan.plo, plan.phi = plo, phi
    plan.nt = (plo + phi) // 128
    slots_b = plo + phi                                  # [L, NBPC]
    off = np.concatenate(([0], np.cumsum(slots_b.reshape(-1))[:-1]))
    plan.blkoff = off.reshape(L, NBPC)
    aux_b = 2 * plan.nt
    aoff = np.concatenate(([0], np.cumsum(aux_b.reshape(-1))[:-1]))
    plan.auxoff = aoff.reshape(L, NBPC)
    plan.tot_slots = int(slots_b.sum())
    plan.tot_aux = int(aux_b.sum())

    S = plan.tot_slots
    # pad slots gather a valid token (idx 0) killed by rl=255 -> sel=0;
    # all-valid keeps dst finite (0 x NaN-garbage would poison the matmul).
    idx_flat = np.zeros((NCORES, S), np.int16)
    rl_flat = np.full((NCORES, S), 255, np.int16)
    val_flat = np.zeros((NCORES, S), np.float32)

    for l in range(L):
        g, idxv, rl, vals = per_l[l]
        order = np.argsort(g, kind="stable")
        g_s = g[order]
        grp_cnt = cnt[l].reshape(-1)
        starts = np.concatenate(([0], np.cumsum(grp_cnt)[:-1]))
        rank = np.arange(len(order)) - starts[g_s]
        k_s = (g_s // 2) % NBPC
        c_s = g_s // (2 * NBPC)
        hi_s = g_s & 1
        slot = (plan.blkoff[l, k_s] + np.where(hi_s == 1, plan.plo[l, k_s], 0)
                + rank)
        idx_flat[c_s, slot] = idxv[order]
        rl_flat[c_s, slot] = rl[order]
        val_flat[c_s, slot] = vals[order]

    idx_w = np.ascontiguousarray(np.tile(
        idx_flat.reshape(NCORES, S // 16, 16).transpose(0, 2, 1), (1, 8, 1)))

    A = plan.tot_aux
    aux = np.zeros((NCORES, 128, A), np.float32)
    valf = np.zeros((NCORES, 128, A // 2), np.float32)
    for l in range(L):
        for k in range(NBPC):
            o, ao = plan.blkoff[l, k], plan.auxoff[l, k]
            nt = plan.nt[l, k]
            s = slice(o, o + nt * 128)
            vt = val_flat[:, s].reshape(NCORES, nt, 128).transpose(0, 2, 1)
            aux[:, :, ao:ao + nt] = rl_flat[:, s].reshape(
                NCORES, nt, 128).transpose(0, 2, 1)
            aux[:, :, ao + nt:ao + 2 * nt] = vt
            valf[:, :, ao // 2:ao // 2 + nt] = vt
    return (plan, idx_w, np.ascontiguousarray(aux.astype(BF16)),
            np.ascontiguousarray(valf))


def _preprocess(phi_indices, phi_values, phi_inverse_indices,
                phi_inverse_values, diagonal_weight_filter):
    diag = np.asarray(diagonal_weight_filter, np.float64)
    rows1, ck1, v1 = [], [], []
    rows2, ck2, v2 = [], [], []
    for l in range(L):
        r1 = phi_inverse_indices[l, 0].astype(np.int64)
        c1 = phi_inverse_indices[l, 1].astype(np.int64)
        rows1.append(r1)
        ck1.append(c1)
        v1.append((np.asarray(phi_inverse_values[l], np.float64) * diag[r1]
                   ).astype(np.float32))
        r2 = phi_indices[l, 0].astype(np.int64)
        c2 = phi_indices[l, 1].astype(np.int64)
        rows2.append(r2)
        ck2.append(_tok2(c2))
        v2.append(np.asarray(phi_values[l], np.float32))
    plan1, idx1, aux1, _ = _preprocess_mat(rows1, ck1, v1)
    plan2, idx2, aux2, val2f = _preprocess_mat(rows2, ck2, v2)
    return plan1, idx1, aux1, plan2, idx2, aux2, val2f


def _chunks(start, padded, valid):
    """Split a run into gather chunks: (slot0, n, reg). All slots hold
    valid idxs (pads = token 0), so reg == n."""
    out = []
    o = 0
    while o < padded:
        n = min(CAP, padded - o)
        out.append((start + o, n, n))
        o += n
    return out


def _build(plan1, plan2, scales, nblocks):
    import concourse.mybir as mybir
    import concourse.tile as tile
    from concourse import bacc

    f32 = mybir.dt.float32
    bf16 = mybir.dt.bfloat16
    i16 = mybir.dt.int16
    eq = mybir.AluOpType.is_equal
    mult = mybir.AluOpType.mult
    AF = mybir.ActivationFunctionType

    NT1MAX = int(plan1.nt.max())
    NT2MAX = int(plan2.nt.max())

    nc = bacc.Bacc("TRN2", target_bir_lowering=False, debug=False,
                   num_devices=NCORES, num_swdge_queues=NQUEUES)
    featsB = nc.dram_tensor("featsB", [L * N, C], bf16, kind="ExternalInput")
    wmat = nc.dram_tensor("wmat", [C, C], bf16, kind="ExternalInput")
    ident = nc.dram_tensor("ident", [C, C], bf16, kind="ExternalInput")
    iotaT = nc.dram_tensor("iotaT", [128, KSEL * 128], bf16,
                           kind="ExternalInput")
    idx1 = nc.dram_tensor("idx1", [128, plan1.tot_slots // 16], i16,
                          kind="ExternalInput")
    idx2 = nc.dram_tensor("idx2", [128, plan2.tot_slots // 16], i16,
                          kind="ExternalInput")
    aux1 = nc.dram_tensor("aux1", [128, plan1.tot_aux], bf16,
                          kind="ExternalInput")
    aux2 = nc.dram_tensor("aux2", [128, plan2.tot_aux], bf16,
                          kind="ExternalInput")
    val2f = nc.dram_tensor("val2f", [128, plan2.tot_aux // 2], f32,
                           kind="ExternalInput")
    outp = nc.dram_tensor("outp", [L, SHARD, C], f32, kind="ExternalOutput")
    debug = os.environ.get("DGW_DEBUG", "0") == "1"
    if debug:
        zdbg = nc.dram_tensor("zdbg", [NCORES * 128, NBPC * C], bf16,
                              kind="ExternalOutput")
        ztabdbg = nc.dram_tensor("ztabdbg", [128, (TBL // 128) * C], bf16,
                                 kind="ExternalOutput")
        dstdbg = nc.dram_tensor("dstdbg", [128, 40 * 128], bf16,
                                kind="ExternalOutput")

    qn = [0]

    def q():
        # queues 0-2: HBM gathers (spmm1). Queue 3: reserved for SBUF-source
        # gathers (spmm2) — concurrent SBUF gathers with different source
        # views cross their streams on HW, so they serialize on one ring.
        qn[0] += 1
        return qn[0] % (NQUEUES - 1)

    with tile.TileContext(nc) as tc:
        with (
            tc.tile_pool(name="const", bufs=1) as constp,
            tc.tile_pool(name="ztab", bufs=1) as zpool,
            tc.tile_pool(name="aux", bufs=6) as auxp,
            tc.tile_pool(name="dst1", bufs=2) as dst1p,
            tc.tile_pool(name="dst2", bufs=2) as dst2p,
            tc.tile_pool(name="sel", bufs=4) as selp,
            tc.tile_pool(name="stg", bufs=6) as stgp,
            tc.tile_pool(name="g2s", bufs=6) as g2sp,
            tc.tile_pool(name="psU", bufs=2, space="PSUM") as psUp,
            tc.tile_pool(name="psZ", bufs=1, space="PSUM") as psZp,
            tc.tile_pool(name="psT", bufs=3, space="PSUM") as psTp,
            tc.tile_pool(name="psO", bufs=2, space="PSUM") as psOp,
            tc.tile_pool(name="dram", bufs=4, space="DRAM") as dramp,
        ):
            w_t = constp.tile([C, C], bf16, name="w_t")
            nc.sync.dma_start(w_t[:], wmat[:])
            id_t = constp.tile([C, C], bf16, name="id_t")
            nc.sync.dma_start(id_t[:], ident[:])
            io_t = constp.tile([128, KSEL * 128], bf16, name="io_t")
            nc.sync.dma_start(io_t[:], iotaT[:])

            def groups(nt):
                g0 = 0
                while g0 < nt:
                    yield g0, min(KSEL, nt - g0)
                    g0 += KSEL

            ztbs = []

            def spmm1_scale(l):
                zshw = dramp.tile([128, NBPC * C], bf16, tag="zshw")
                ztbw = dramp.tile([NCORES * 128, NBPC * C], bf16, tag="ztbw",
                                  addr_space="Shared")
                ztbs.append(ztbw)
                for b in range(nblocks):
                    nt = int(plan1.nt[l, b])
                    o = int(plan1.blkoff[l, b])
                    ao = int(plan1.auxoff[l, b])
                    it = auxp.tile([128, nt * 8], i16, tag="idx1t")
                    nc.sync.dma_start(
                        it[:], idx1[:, o // 16:(o + nt * 128) // 16])
                    vt = auxp.tile([128, 2 * nt], bf16, tag="vr1")
                    nc.sync.dma_start(vt[:], aux1[:, ao:ao + 2 * nt])
                    dst = dst1p.tile([128, NT1MAX, C], bf16, tag="dst1")
                    runs = (_chunks(0, int(plan1.plo[l, b]),
                                    int(plan1.maxlo[l, b]), ) +
                            _chunks(int(plan1.plo[l, b]),
                                    int(plan1.phi[l, b]),
                                    int(plan1.maxhi[l, b])))
                    nlo = int(plan1.plo[l, b])
                    for (s0, n, reg) in runs:
                        src = (featsB[l * N:l * N + HALF, :] if s0 < nlo
                               else featsB[l * N + HALF:(l + 1) * N, :])
                        nc.gpsimd.dma_gather(
                            dst[:, s0 // 128:(s0 + n) // 128, :], src,
                            it[:, s0 // 16:(s0 + n) // 16], n, reg, C,
                            single_packet=False, queue_num=q())
                    psU = psUp.tile([128, 128], f32)
                    for g0, K in groups(nt):
                        sel = selp.tile([128, KSEL * 128], bf16, tag="sel")
                        s3 = sel[:, :K * 128].rearrange(
                            "p (k r) -> p k r", k=K)
                        rl_b = vt[:, g0:g0 + K].to_broadcast([128, K, 128])
                        v_b = vt[:, nt + g0:nt + g0 + K].to_broadcast(
                            [128, K, 128])
                        io_v = io_t[:, :K * 128].rearrange(
                            "p (k r) -> p k r", k=K)
                        nc.vector.tensor_tensor(out=s3, in0=rl_b, in1=io_v,
                                                op=eq)
                        nc.vector.tensor_tensor(out=s3, in0=s3, in1=v_b,
                                                op=mult)
                        for k in range(K):
                            t = g0 + k
                            nc.tensor.matmul(
                                psU[:], lhsT=dst[:, t, :],
                                rhs=sel[:, k * 128:(k + 1) * 128],
                                start=(t == 0), stop=(t == nt - 1))
                    ut = stgp.tile([128, 128], bf16, tag="ut")
                    nc.scalar.activation(ut[:], psU[:], AF.Copy)
                    psZ = psZp.tile([128, 128], f32)
                    nc.tensor.matmul(psZ[:], lhsT=ut[:], rhs=w_t[:],
                                     start=True, stop=True)
                    zt = stgp.tile([128, 128], bf16, tag="zt")
                    nc.scalar.activation(zt[:], psZ[:], AF.Copy)
                    nc.sync.dma_start(zshw[:, b * C:(b + 1) * C], zt[:])
                nc.gpsimd.collective_compute(
                    "AllGather", mybir.AluOpType.bypass,
                    replica_groups=[list(range(NCORES))],
                    ins=[zshw.opt()], outs=[ztbw.opt()])

            def spmm2_scale(l):
                ztbw = ztbs[l]
                ztab = zpool.tile([128, (TBL // 128) * C], bf16, tag="ztab")
                for csec in range(NCORES):
                    nc.gpsimd.dma_start(
                        ztab[:, csec * NBPC * C:(csec + 1) * NBPC * C],
                        ztbw[csec * 128:(csec + 1) * 128, :])
                if debug and l == 0:
                    nc.sync.dma_start(zdbg[:], ztbw[:])
                    nc.sync.dma_start(ztabdbg[:], ztab[:])
                src_lo = ztab[:, :256 * C]
                src_hi = ztab[:, 256 * C:]
                border = range(nblocks)
                if os.environ.get("DGW_REV2", "0") == "1":
                    border = range(nblocks - 1, -1, -1)
                for b in border:
                    nt = int(plan2.nt[l, b])
                    o = int(plan2.blkoff[l, b])
                    ao = int(plan2.auxoff[l, b])
                    it = auxp.tile([128, nt * 8], i16, tag="idx2t")
                    nc.sync.dma_start(
                        it[:], idx2[:, o // 16:(o + nt * 128) // 16])
                    vt = auxp.tile([128, 2 * nt], bf16, tag="vr2")
                    nc.sync.dma_start(vt[:], aux2[:, ao:ao + 2 * nt])
                    v2t = auxp.tile([128, nt], f32, tag="v2f")
                    nc.sync.dma_start(v2t[:], val2f[:, ao // 2:ao // 2 + nt])
                    dst = dst2p.tile([128, 1, NT2MAX * 128], bf16, tag="dst2")
                    runs = (_chunks(0, int(plan2.plo[l, b]),
                                    int(plan2.maxlo[l, b])) +
                            _chunks(int(plan2.plo[l, b]),
                                    int(plan2.phi[l, b]),
                                    int(plan2.maxhi[l, b])))
                    nlo = int(plan2.plo[l, b])
                    for (s0, n, reg) in runs:
                        src = src_lo if s0 < nlo else src_hi
                        nc.gpsimd.dma_gather(
                            dst[:, :, s0:s0 + n], src,
                            it[:, s0 // 16:(s0 + n) // 16], n, reg, C,
                            transpose=True, single_packet=False,
                            queue_num=NQUEUES - 1,
                            sbuf_tokens_per_rank=128,
                            sbuf_free_dim_per_rank=256,
                            sbuf_free_dim_pad_per_rank=0,
                            sbuf_byte_offset=0)
                    if debug and l == 0 and b == 0:
                        nc.sync.dma_start(
                            dstdbg[:, :nt * 128], dst[:, 0, :nt * 128])
                    psO = psOp.tile([128, 128], f32)
                    for g0, K in groups(nt):
                        sel = selp.tile([128, KSEL * 128], bf16, tag="sel2")
                        s3 = sel[:, :K * 128].rearrange(
                            "p (k r) -> p k r", k=K)
                        rl_b = vt[:, g0:g0 + K].to_broadcast([128, K, 128])
                        io_v = io_t[:, :K * 128].rearrange(
                            "p (k r) -> p k r", k=K)
                        nc.vector.tensor_tensor(out=s3, in0=rl_b, in1=io_v,
                                                op=eq)
                        for k in range(K):
                            t = g0 + k
                            psT = psTp.tile([128, 128], f32)
                            nc.tensor.matmul(
                                psT[:], lhsT=dst[:, 0, t * 128:(t + 1) * 128],
                                rhs=id_t[:], start=True, stop=True)
                            g2 = g2sp.tile([128, 128], bf16, tag="g2")
                            nc.scalar.activation(
                                g2[:], psT[:], AF.Copy,
                                scale=v2t[:, t:t + 1])
                            nc.tensor.matmul(
                                psO[:], lhsT=sel[:, k * 128:(k + 1) * 128],
                                rhs=g2[:],
                                start=(t == 0), stop=(t == nt - 1))
                    ot = stgp.tile([128, 128], f32, tag="ot")
                    nc.vector.tensor_copy(ot[:], psO[:])
                    nc.sync.dma_start(outp[l, b * BLK:(b + 1) * BLK, :], ot[:])

            # Interleave scales so spmm2's PE-heavy work spreads through the
            # gather-gen timeline instead of all landing at the end.
            done1 = done2 = 0
            while done2 < scales:
                if done1 < scales:
                    spmm1_scale(done1)
                    done1 += 1
                if done1 >= min(2, scales) and done2 < done1:
                    spmm2_scale(done2)
                    done2 += 1
    nc.compile()
    return nc


def kernel(**inputs):
    _install_hook_stub()
    from concourse.bass_utils import run_bass_kernel_spmd

    feats = np.asarray(inputs["features"], np.float32)        # [N, L, C]
    featsB = np.ascontiguousarray(
        feats.transpose(1, 0, 2).reshape(L * N, C)).astype(BF16)
    wmat = np.asarray(inputs["weight_matrix"], np.float32).astype(BF16)

    plan1, idx1, aux1, plan2, idx2, aux2, val2f = _preprocess(
        np.asarray(inputs["phi_indices"]), np.asarray(inputs["phi_values"]),
        np.asarray(inputs["phi_inverse_indices"]),
        np.asarray(inputs["phi_inverse_values"]),
        np.asarray(inputs["diagonal_weight_filter"]))

    scales = int(os.environ.get("DGW_SCALES", L))
    nblocks = int(os.environ.get("DGW_BLOCKS", NBPC))
    nc = _build(plan1, plan2, scales, nblocks)

    ident = np.eye(C, dtype=np.float32).astype(BF16)
    iotaT = np.ascontiguousarray(np.tile(
        np.arange(128, dtype=np.float32)[None, :], (128, KSEL))).astype(BF16)

    in_maps = []
    for c in range(NCORES):
        in_maps.append(dict(
            featsB=featsB, wmat=wmat, ident=ident, iotaT=iotaT,
            idx1=np.ascontiguousarray(idx1[c]),
            idx2=np.ascontiguousarray(idx2[c]),
            aux1=np.ascontiguousarray(aux1[c]),
            aux2=np.ascontiguousarray(aux2[c]),
            val2f=np.ascontiguousarray(val2f[c])))
    res = run_bass_kernel_spmd(nc, in_maps, core_ids=list(range(NCORES)))
    kernel.last_results = res

    shards = np.stack([res.results[c]["outp"] for c in range(NCORES)])
    blocks = shards.reshape(NCORES, L, NBPC, BLK, C).transpose(2, 0, 3, 1, 4)
    out = blocks.reshape(NBPC * NCORES * BLK, L, C)[:N]
    return np.ascontiguousarray(out)



# revision 6
# speedup vs baseline: 1.5949x; 1.5949x over previous
"""Trainium2 Bass kernel for nn_DenseGraphWaveletLayer (v5).

out[:, l, :] = phi_l @ diag(theta) @ phi_inv_l @ (features[:, l, :] @ W)

v5 strategy (8 cores SPMD, one program, per-core data):
  - Both spmms are HBM dma_gather (transpose=False, 256B rows) + one-hot
    sel matmuls. spmm1 gathers bf16 feature rows; its edge values carry
    diag(theta) folded on host. spmm2 gathers bf16 z rows from the
    AllGather'd shared-DRAM z table (no SBUF token table, no transpose
    matmuls, no per-tile scalar copies).
  - Slot order per (l, mat): half-major (lo run, hi run), blocks 128-padded
    inside each run, so gathers span many blocks: chunks of up to 8192
    idxs on 4 round-robin SWDGE queues, double-buffered dst tiles.
  - sel = (rl == iota) * val built on DVE in KSEL=16 tile groups; pads use
    rl=255/val=0 over a valid dup token so everything stays finite.
  - aux (rl/val) loaded once per (l, mat); idx loaded per chunk.
  - psU accumulates U^T per 128-row block; z = U @ W via lhsT=U^T; z rows
    staged per-scale in SBUF and written with one DMA before AllGather.
"""

import os
import sys
import types

import numpy as np
import ml_dtypes

BF16 = ml_dtypes.bfloat16

N = 50000
L = 4
C = 128
NCORES = 8
BLK = 128
NB_TOT = (N + BLK - 1) // BLK            # 391
NBPC = (NB_TOT + NCORES - 1) // NCORES   # 49
SHARD = NBPC * BLK                       # 6272
TBL = NCORES * SHARD                     # 50176
HALF = 32768
CAP = int(os.environ.get("DGW_CAP", 4096))
KSEL = int(os.environ.get("DGW_KSEL", 16))
NQUEUES = 4
MINCNT = 16


def _install_hook_stub():
    try:
        import antenv
    except ImportError:
        return
    try:
        from antenv import axon_hooks  # noqa: F401
        return
    except ImportError:
        pass
    mod = types.ModuleType("antenv.axon_hooks")
    mod._hook = None
    mod.set_axon_ntff_profile_hook = lambda h: setattr(mod, "_hook", h)
    mod.get_axon_ntff_profile_hook = lambda: mod._hook
    sys.modules["antenv.axon_hooks"] = mod
    antenv.axon_hooks = mod


def _tok2(cols):
    """z-table token id for spmm2: row (core*128 + rl) * NBPC + k."""
    cblk = cols >> 7
    rl = cols & 127
    core = cblk % NCORES
    k = cblk // NCORES
    return (core * 128 + rl) * NBPC + k


class MatPlan:
    """Per-(l, half, block) slot layout for one sparse matrix (all cores)."""

    def __init__(self):
        self.slots = None    # [L, 2, NBPC] padded slot counts (mult of 128)
        self.segoff = None   # [L, 2, NBPC] slot offset of segment
        self.auxoff = None   # [L, 2, NBPC] aux col offset (rl cols start)
        self.nt = None       # [L, 2, NBPC] tiles per segment
        self.tot_slots = 0
        self.tot_aux = 0


def _preprocess_mat(rows_l, ckey_l, vals_l):
    """rows/ckey/vals: lists of L arrays (full edge sets).

    Returns (plan, idx [8,128,S/16], aux [8,128,A])."""
    plan = MatPlan()
    cnt = np.zeros((L, NCORES, 2, NBPC), np.int64)
    per_l = []
    for l in range(L):
        rows, ckey, vals = rows_l[l], ckey_l[l], vals_l[l]
        core = (rows >> 7) % NCORES
        k = rows >> 10
        rl = (rows & 127).astype(np.int16)
        hi = (ckey >= HALF).astype(np.int64)
        idxv = (ckey - HALF * hi).astype(np.int16)
        g = ((core * 2 + hi) * NBPC + k).astype(np.int64)
        cnt[l] = np.bincount(g, minlength=NCORES * 2 * NBPC).reshape(
            NCORES, 2, NBPC)
        per_l.append((g, idxv, rl, vals.astype(np.float32)))

    maxc = np.maximum(cnt.max(axis=1), MINCNT)            # [L, 2, NBPC]
    plan.slots = ((maxc + 127) // 128) * 128
    plan.nt = plan.slots // 128
    flat = plan.slots.reshape(-1)
    off = np.concatenate(([0], np.cumsum(flat)[:-1]))
    plan.segoff = off.reshape(L, 2, NBPC)
    aux_b = 2 * plan.nt
    aoff = np.concatenate(([0], np.cumsum(aux_b.reshape(-1))[:-1]))
    plan.auxoff = aoff.reshape(L, 2, NBPC)
    plan.tot_slots = int(flat.sum())
    plan.tot_aux = int(aux_b.sum())

    S = plan.tot_slots
    # pad slots gather a valid token (idx 0) killed by rl=255 -> sel=0.
    idx_flat = np.zeros((NCORES, S), np.int16)
    rl_flat = np.full((NCORES, S), 255, np.int16)
    val_flat = np.zeros((NCORES, S), np.float32)

    for l in range(L):
        g, idxv, rl, vals = per_l[l]
        order = np.argsort(g, kind="stable")
        g_s = g[order]
        grp_cnt = cnt[l].reshape(-1)
        starts = np.concatenate(([0], np.cumsum(grp_cnt)[:-1]))
        rank = np.arange(len(order)) - starts[g_s]
        c_s = g_s // (2 * NBPC)
        hi_s = (g_s // NBPC) % 2
        k_s = g_s % NBPC
        slot = plan.segoff[l, hi_s, k_s] + rank
        idx_flat[c_s, slot] = idxv[order]
        rl_flat[c_s, slot] = rl[order]
        val_flat[c_s, slot] = vals[order]

    idx_w = np.ascontiguousarray(np.tile(
        idx_flat.reshape(NCORES, S // 16, 16).transpose(0, 2, 1), (1, 8, 1)))

    A = plan.tot_aux
    aux = np.zeros((NCORES, 128, A), np.float32)
    for l in range(L):
        for h in range(2):
            for k in range(NBPC):
                o = plan.segoff[l, h, k]
                ao = plan.auxoff[l, h, k]
                nt = plan.nt[l, h, k]
                s = slice(o, o + nt * 128)
                aux[:, :, ao:ao + nt] = rl_flat[:, s].reshape(
                    NCORES, nt, 128).transpose(0, 2, 1)
                aux[:, :, ao + nt:ao + 2 * nt] = val_flat[:, s].reshape(
                    NCORES, nt, 128).transpose(0, 2, 1)
    return plan, idx_w, np.ascontiguousarray(aux.astype(BF16))


def _preprocess(phi_indices, phi_values, phi_inverse_indices,
                phi_inverse_values, diagonal_weight_filter):
    diag = np.asarray(diagonal_weight_filter, np.float64)
    rows1, ck1, v1 = [], [], []
    rows2, ck2, v2 = [], [], []
    for l in range(L):
        r1 = phi_inverse_indices[l, 0].astype(np.int64)
        c1 = phi_inverse_indices[l, 1].astype(np.int64)
        rows1.append(r1)
        ck1.append(c1)
        v1.append((np.asarray(phi_inverse_values[l], np.float64) * diag[r1]
                   ).astype(np.float32))
        r2 = phi_indices[l, 0].astype(np.int64)
        c2 = phi_indices[l, 1].astype(np.int64)
        rows2.append(r2)
        ck2.append(_tok2(c2))
        v2.append(np.asarray(phi_values[l], np.float32))
    plan1, idx1, aux1 = _preprocess_mat(rows1, ck1, v1)
    plan2, idx2, aux2 = _preprocess_mat(rows2, ck2, v2)
    return plan1, idx1, aux1, plan2, idx2, aux2


def _build(plan1, plan2, scales):
    import concourse.mybir as mybir
    import concourse.tile as tile
    from concourse import bacc

    f32 = mybir.dt.float32
    bf16 = mybir.dt.bfloat16
    i16 = mybir.dt.int16
    eq = mybir.AluOpType.is_equal
    mult = mybir.AluOpType.mult
    AF = mybir.ActivationFunctionType

    nc = bacc.Bacc("TRN2", target_bir_lowering=False, debug=False,
                   num_devices=NCORES, num_swdge_queues=NQUEUES)
    featsB = nc.dram_tensor("featsB", [L * N, C], bf16, kind="ExternalInput")
    wmat = nc.dram_tensor("wmat", [C, C], bf16, kind="ExternalInput")
    iotaT = nc.dram_tensor("iotaT", [128, KSEL * 128], bf16,
                           kind="ExternalInput")
    idx1 = nc.dram_tensor("idx1", [128, plan1.tot_slots // 16], i16,
                          kind="ExternalInput")
    idx2 = nc.dram_tensor("idx2", [128, plan2.tot_slots // 16], i16,
                          kind="ExternalInput")
    aux1 = nc.dram_tensor("aux1", [128, plan1.tot_aux], bf16,
                          kind="ExternalInput")
    aux2 = nc.dram_tensor("aux2", [128, plan2.tot_aux], bf16,
                          kind="ExternalInput")
    outp = nc.dram_tensor("outp", [L, SHARD, C], f32, kind="ExternalOutput")

    qn = [0]

    def q():
        qn[0] += 1
        return qn[0] % NQUEUES

    with tile.TileContext(nc) as tc:
        with (
            tc.tile_pool(name="const", bufs=1) as constp,
            tc.tile_pool(name="aux", bufs=3) as auxp,
            tc.tile_pool(name="idx", bufs=8) as idxp,
            tc.tile_pool(name="dst1", bufs=6) as dst1p,
            tc.tile_pool(name="dst2", bufs=6) as dst2p,
            tc.tile_pool(name="sel", bufs=6) as selp,
            tc.tile_pool(name="stg", bufs=4) as stgp,
            tc.tile_pool(name="zsb", bufs=2) as zsbp,
            tc.tile_pool(name="psU", bufs=2, space="PSUM") as psUp,
            tc.tile_pool(name="psZ", bufs=1, space="PSUM") as psZp,
            tc.tile_pool(name="psO", bufs=2, space="PSUM") as psOp,
            tc.tile_pool(name="dram", bufs=4, space="DRAM") as dramp,
        ):
            w_t = constp.tile([C, C], bf16, name="w_t")
            nc.sync.dma_start(w_t[:], wmat[:])
            io_t = constp.tile([128, KSEL * 128], bf16, name="io_t")
            nc.sync.dma_start(io_t[:], iotaT[:])

            ztbs = []

            def emit_mat(plan, idxT, auxT, dstp, l, srcs, consume_block):
                """Gathers (chunk-interleaved lo/hi) + per-block matmul
                consumption for one (l, mat). consume_block(b, tiles) is
                called once per block with the list of (tile, rank) slot
                tiles in order; it must do the PSUM accumulation + output.
                srcs = (src_lo, src_hi) dram APs."""
                ao0 = int(plan.auxoff[l, 0, 0])
                a_end = (int(plan.auxoff[l, 1, NBPC - 1])
                         + 2 * int(plan.nt[l, 1, NBPC - 1]))
                vt = auxp.tile([128, a_end - ao0], bf16, tag="aux")
                nc.sync.dma_start(vt[:], auxT[:, ao0:a_end])

                runs = []
                for h in range(2):
                    s0 = int(plan.segoff[l, h, 0])
                    ln = (int(plan.segoff[l, h, NBPC - 1])
                          + int(plan.slots[l, h, NBPC - 1]) - s0)
                    chunks = []
                    o = 0
                    while o < ln:
                        n = min(CAP, ln - o)
                        chunks.append((s0 + o, n))
                        o += n
                    runs.append(chunks)

                tiles = {}          # (h, i) -> (tile, slot0)
                covered = [0, 0]    # slots gathered per half (absolute end)
                nextb = [0]

                def emit_ready():
                    while nextb[0] < NBPC:
                        b = nextb[0]
                        ends = [int(plan.segoff[l, h, b])
                                + int(plan.slots[l, h, b]) for h in range(2)]
                        if covered[0] < ends[0] or covered[1] < ends[1]:
                            return
                        # collect this block's slot tiles in order lo, hi
                        tl = []
                        for h in range(2):
                            seg0 = int(plan.segoff[l, h, b])
                            nt = int(plan.nt[l, h, b])
                            run0 = int(plan.segoff[l, h, 0])
                            for t in range(nt):
                                s = seg0 + t * 128 - run0
                                ti, loc = s // CAP, (s % CAP) // 128
                                tl.append((tiles[(h, ti)][0], loc,
                                           (h, b, t)))
                        consume_block(b, tl, vt, ao0)
                        nextb[0] += 1

                # Emit chunks balanced by BLOCK coverage per half — the
                # halves are unevenly sized (lo:hi ~ 2:1) and GpSimd runs
                # in-order, so index-paired emission deadlocks pool reuse.
                def blocks_done(h):
                    bc = 0
                    while bc < NBPC and (int(plan.segoff[l, h, bc])
                                         + int(plan.slots[l, h, bc])
                                         <= covered[h]):
                        bc += 1
                    return bc

                ci = [0, 0]
                while ci[0] < len(runs[0]) or ci[1] < len(runs[1]):
                    if ci[0] >= len(runs[0]):
                        h = 1
                    elif ci[1] >= len(runs[1]):
                        h = 0
                    else:
                        h = 0 if blocks_done(0) <= blocks_done(1) else 1
                    s0, n = runs[h][ci[h]]
                    dst = dstp.tile([128, CAP // 128, C], bf16, tag="dst")
                    it = idxp.tile([128, CAP // 16], i16, tag="idx")
                    nc.sync.dma_start(
                        it[:, :n // 16], idxT[:, s0 // 16:(s0 + n) // 16])
                    nc.gpsimd.dma_gather(
                        dst[:, :n // 128, :], srcs[h],
                        it[:, :n // 16], n, n, C,
                        single_packet=False, queue_num=q())
                    tiles[(h, ci[h])] = (dst, s0)
                    covered[h] = s0 + n
                    ci[h] += 1
                    emit_ready()
                emit_ready()
                assert nextb[0] == NBPC

            def sel_groups(tl, vt, ao0, plan, l, use_val=True):
                """Yield (sel_tile, k, tile, loc) quadruples: sel columns
                aligned with consecutive tiles of one (h, b) segment."""
                i = 0
                while i < len(tl):
                    h, b, t0 = tl[i][2]
                    nt = int(plan.nt[l, h, b])
                    K = min(KSEL, nt - t0)
                    ao = int(plan.auxoff[l, h, b]) - ao0
                    sel = selp.tile([128, KSEL * 128], bf16, tag="sel")
                    s3 = sel[:, :K * 128].rearrange("p (k r) -> p k r", k=K)
                    rl_b = vt[:, ao + t0:ao + t0 + K].to_broadcast(
                        [128, K, 128])
                    io_v = io_t[:, :K * 128].rearrange(
                        "p (k r) -> p k r", k=K)
                    nc.vector.tensor_tensor(out=s3, in0=rl_b, in1=io_v,
                                            op=eq)
                    if use_val:
                        v_b = vt[:, ao + nt + t0:ao + nt + t0 + K
                                 ].to_broadcast([128, K, 128])
                        nc.vector.tensor_tensor(out=s3, in0=s3, in1=v_b,
                                                op=mult)
                    for k in range(K):
                        yield sel, k, tl[i + k][0], tl[i + k][1]
                    i += K

            def spmm1_scale(l):
                zshw = dramp.tile([128, NBPC * C], bf16, tag="zshw")
                ztbw = dramp.tile([TBL, C], bf16, tag="ztbw",
                                  addr_space="Shared")
                ztbs.append(ztbw)
                zsb = zsbp.tile([128, NBPC * C], bf16, tag="zsb")
                src_lo = featsB[l * N:l * N + HALF, :]
                src_hi = featsB[l * N + HALF:(l + 1) * N, :]

                def consume(b, tl, vt, ao0):
                    ntot = len(tl)
                    psU = psUp.tile([128, 128], f32)
                    j = 0
                    for sel, k, dt, loc in sel_groups(tl, vt, ao0, plan1, l):
                        nc.tensor.matmul(
                            psU[:], lhsT=dt[:, loc, :],
                            rhs=sel[:, k * 128:(k + 1) * 128],
                            start=(j == 0), stop=(j == ntot - 1))
                        j += 1
                    ut = stgp.tile([128, 128], bf16, tag="ut")
                    nc.scalar.activation(ut[:], psU[:], AF.Copy)
                    psZ = psZp.tile([128, 128], f32)
                    nc.tensor.matmul(psZ[:], lhsT=ut[:], rhs=w_t[:],
                                     start=True, stop=True)
                    nc.scalar.activation(zsb[:, b * C:(b + 1) * C], psZ[:],
                                         AF.Copy)

                emit_mat(plan1, idx1, aux1, dst1p, l, (src_lo, src_hi),
                         consume)
                nc.scalar.dma_start(zshw[:], zsb[:])
                nc.gpsimd.collective_compute(
                    "AllGather", mybir.AluOpType.bypass,
                    replica_groups=[list(range(NCORES))],
                    ins=[zshw.opt()], outs=[ztbw.opt()])

            def spmm2_scale(l):
                ztbw = ztbs[l]
                src_lo = ztbw[:HALF, :]
                src_hi = ztbw[HALF:, :]

                def consume(b, tl, vt, ao0):
                    ntot = len(tl)
                    psO = psOp.tile([128, 128], f32)
                    j = 0
                    for sel, k, dt, loc in sel_groups(tl, vt, ao0, plan2, l):
                        nc.tensor.matmul(
                            psO[:], lhsT=sel[:, k * 128:(k + 1) * 128],
                            rhs=dt[:, loc, :],
                            start=(j == 0), stop=(j == ntot - 1))
                        j += 1
                    ot = stgp.tile([128, 128], f32, tag="ot")
                    nc.vector.tensor_copy(ot[:], psO[:])
                    nc.scalar.dma_start(outp[l, b * BLK:(b + 1) * BLK, :],
                                        ot[:])

                emit_mat(plan2, idx2, aux2, dst2p, l, (src_lo, src_hi),
                         consume)

            done1 = done2 = 0
            while done2 < scales:
                if done1 < scales:
                    spmm1_scale(done1)
                    done1 += 1
                if done1 >= min(2, scales) and done2 < done1:
                    spmm2_scale(done2)
                    done2 += 1
    nc.compile()
    return nc


def kernel(**inputs):
    _install_hook_stub()
    from concourse.bass_utils import run_bass_kernel_spmd

    feats = np.asarray(inputs["features"], np.float32)        # [N, L, C]
    featsB = np.ascontiguousarray(
        feats.transpose(1, 0, 2).reshape(L * N, C)).astype(BF16)
    wmat = np.asarray(inputs["weight_matrix"], np.float32).astype(BF16)

    plan1, idx1, aux1, plan2, idx2, aux2 = _preprocess(
        np.asarray(inputs["phi_indices"]), np.asarray(inputs["phi_values"]),
        np.asarray(inputs["phi_inverse_indices"]),
        np.asarray(inputs["phi_inverse_values"]),
        np.asarray(inputs["diagonal_weight_filter"]))

    scales = int(os.environ.get("DGW_SCALES", L))
    nc = _build(plan1, plan2, scales)

    iotaT = np.ascontiguousarray(np.tile(
        np.arange(128, dtype=np.float32)[None, :], (128, KSEL))).astype(BF16)

    in_maps = []
    for c in range(NCORES):
        in_maps.append(dict(
            featsB=featsB, wmat=wmat, iotaT=iotaT,
            idx1=np.ascontiguousarray(idx1[c]),
            idx2=np.ascontiguousarray(idx2[c]),
            aux1=np.ascontiguousarray(aux1[c]),
            aux2=np.ascontiguousarray(aux2[c])))
    res = run_bass_kernel_spmd(nc, in_maps, core_ids=list(range(NCORES)))
    kernel.last_results = res

    shards = np.stack([res.results[c]["outp"] for c in range(NCORES)])
    blocks = shards.reshape(NCORES, L, NBPC, BLK, C).transpose(2, 0, 3, 1, 4)
    out = blocks.reshape(NBPC * NCORES * BLK, L, C)[:N]
    return np.ascontiguousarray(out)


# revision 7
# speedup vs baseline: 1.8102x; 1.1350x over previous
"""Trainium2 Bass kernel for nn_DenseGraphWaveletLayer (v6).

out[:, l, :] = phi_l @ diag(theta) @ phi_inv_l @ (features[:, l, :] @ W)

v6 strategy (8 cores SPMD, one program, per-core data):
  - Both spmms are HBM dma_gather (transpose=False, 256B rows) + one-hot
    sel matmuls. spmm1 gathers bf16 feature rows; its edge values carry
    diag(theta) folded on host. spmm2 gathers bf16 z rows from the
    AllGather'd shared-DRAM z table.
  - sel is built per 64-row WINDOW (each 128-row block = 2 windows, with
    slots row-sorted and window-partitioned on host), halving the DVE
    one-hot build: sel = (rl' == iota64) * val over [slot, 64] only.
    Matmuls write psU[:, w*64:(w+1)*64] (spmm1) / psO[w*64:.., :] (spmm2)
    as two independent accumulation chains per block.
  - Slot order per (l, mat): half-major (lo run, hi run); gathers span
    blocks in chunks of up to DGW_CAP idxs on 4 round-robin SWDGE queues,
    emitted balanced by per-half block coverage.
  - aux (rl'/val) loaded once per (l, mat); idx loaded per chunk; pads use
    rl'=200/val=0 over a valid dup token so everything stays finite.
"""

import os
import sys
import types

import numpy as np
import ml_dtypes

BF16 = ml_dtypes.bfloat16

N = 50000
L = 4
C = 128
NCORES = 8
BLK = 128
NB_TOT = (N + BLK - 1) // BLK            # 391
NBPC = (NB_TOT + NCORES - 1) // NCORES   # 49
SHARD = NBPC * BLK                       # 6272
TBL = NCORES * SHARD                     # 50176
HALF = 32768
W = 64                                   # sel row-window width
CAP = int(os.environ.get("DGW_CAP", 4096))
KSEL = int(os.environ.get("DGW_KSEL", 16))
NQUEUES = 4
MINCNT = 16


def _install_hook_stub():
    try:
        import antenv
    except ImportError:
        return
    try:
        from antenv import axon_hooks  # noqa: F401
        return
    except ImportError:
        pass
    mod = types.ModuleType("antenv.axon_hooks")
    mod._hook = None
    mod.set_axon_ntff_profile_hook = lambda h: setattr(mod, "_hook", h)
    mod.get_axon_ntff_profile_hook = lambda: mod._hook
    sys.modules["antenv.axon_hooks"] = mod
    antenv.axon_hooks = mod


def _tok2(cols):
    """z-table token id for spmm2: row (core*128 + rl) * NBPC + k."""
    cblk = cols >> 7
    rl = cols & 127
    core = cblk % NCORES
    k = cblk // NCORES
    return (core * 128 + rl) * NBPC + k


class MatPlan:
    """Per-(l, half, block, win) slot layout for one sparse matrix."""

    def __init__(self):
        self.slots = None    # [L, 2, NBPC, 2] padded counts (mult of 128)
        self.segoff = None   # [L, 2, NBPC, 2] slot offset of sub-segment
        self.auxoff = None   # [L, 2, NBPC, 2] aux col offset
        self.nt = None       # [L, 2, NBPC, 2] tiles per sub-segment
        self.tot_slots = 0
        self.tot_aux = 0


def _preprocess_mat(rows_l, ckey_l, vals_l):
    """rows/ckey/vals: lists of L arrays (full edge sets).

    Returns (plan, idx [8,128,S/16], aux [8,128,A])."""
    plan = MatPlan()
    NW = 2
    cnt = np.zeros((L, NCORES, 2, NBPC, NW), np.int64)
    per_l = []
    for l in range(L):
        rows, ckey, vals = rows_l[l], ckey_l[l], vals_l[l]
        core = (rows >> 7) % NCORES
        k = rows >> 10
        rl = (rows & 127).astype(np.int64)
        w = rl >> 6
        hi = (ckey >= HALF).astype(np.int64)
        idxv = (ckey - HALF * hi).astype(np.int16)
        g = (((core * 2 + hi) * NBPC + k) * NW + w).astype(np.int64)
        cnt[l] = np.bincount(g, minlength=NCORES * 2 * NBPC * NW).reshape(
            NCORES, 2, NBPC, NW)
        per_l.append((g, idxv, (rl & 63).astype(np.int16),
                      vals.astype(np.float32)))

    maxc = np.maximum(cnt.max(axis=1), MINCNT)          # [L, 2, NBPC, NW]
    plan.slots = ((maxc + 127) // 128) * 128
    plan.nt = plan.slots // 128
    flat = plan.slots.reshape(-1)
    off = np.concatenate(([0], np.cumsum(flat)[:-1]))
    plan.segoff = off.reshape(L, 2, NBPC, NW)
    aux_b = 2 * plan.nt
    aoff = np.concatenate(([0], np.cumsum(aux_b.reshape(-1))[:-1]))
    plan.auxoff = aoff.reshape(L, 2, NBPC, NW)
    plan.tot_slots = int(flat.sum())
    plan.tot_aux = int(aux_b.sum())

    S = plan.tot_slots
    # pad slots gather a valid token (idx 0) killed by rl'=200 -> sel=0.
    idx_flat = np.zeros((NCORES, S), np.int16)
    rl_flat = np.full((NCORES, S), 200, np.int16)
    val_flat = np.zeros((NCORES, S), np.float32)

    for l in range(L):
        g, idxv, rlw, vals = per_l[l]
        order = np.argsort(g, kind="stable")
        g_s = g[order]
        grp_cnt = cnt[l].reshape(-1)
        starts = np.concatenate(([0], np.cumsum(grp_cnt)[:-1]))
        rank = np.arange(len(order)) - starts[g_s]
        c_s = g_s // (2 * NBPC * NW)
        hi_s = (g_s // (NBPC * NW)) % 2
        k_s = (g_s // NW) % NBPC
        w_s = g_s % NW
        slot = plan.segoff[l, hi_s, k_s, w_s] + rank
        idx_flat[c_s, slot] = idxv[order]
        rl_flat[c_s, slot] = rlw[order]
        val_flat[c_s, slot] = vals[order]

    idx_w = np.ascontiguousarray(np.tile(
        idx_flat.reshape(NCORES, S // 16, 16).transpose(0, 2, 1), (1, 8, 1)))

    A = plan.tot_aux
    aux = np.zeros((NCORES, 128, A), np.float32)
    for l in range(L):
        for h in range(2):
            for k in range(NBPC):
                for w in range(NW):
                    o = plan.segoff[l, h, k, w]
                    ao = plan.auxoff[l, h, k, w]
                    nt = plan.nt[l, h, k, w]
                    s = slice(o, o + nt * 128)
                    aux[:, :, ao:ao + nt] = rl_flat[:, s].reshape(
                        NCORES, nt, 128).transpose(0, 2, 1)
                    aux[:, :, ao + nt:ao + 2 * nt] = val_flat[:, s].reshape(
                        NCORES, nt, 128).transpose(0, 2, 1)
    return plan, idx_w, np.ascontiguousarray(aux.astype(BF16))


def _preprocess(phi_indices, phi_values, phi_inverse_indices,
                phi_inverse_values, diagonal_weight_filter):
    diag = np.asarray(diagonal_weight_filter, np.float64)
    rows1, ck1, v1 = [], [], []
    rows2, ck2, v2 = [], [], []
    for l in range(L):
        r1 = phi_inverse_indices[l, 0].astype(np.int64)
        c1 = phi_inverse_indices[l, 1].astype(np.int64)
        rows1.append(r1)
        ck1.append(c1)
        v1.append((np.asarray(phi_inverse_values[l], np.float64) * diag[r1]
                   ).astype(np.float32))
        r2 = phi_indices[l, 0].astype(np.int64)
        c2 = phi_indices[l, 1].astype(np.int64)
        rows2.append(r2)
        ck2.append(_tok2(c2))
        v2.append(np.asarray(phi_values[l], np.float32))
    plan1, idx1, aux1 = _preprocess_mat(rows1, ck1, v1)
    plan2, idx2, aux2 = _preprocess_mat(rows2, ck2, v2)
    return plan1, idx1, aux1, plan2, idx2, aux2


def _build(plan1, plan2, scales):
    import concourse.mybir as mybir
    import concourse.tile as tile
    from concourse import bacc

    f32 = mybir.dt.float32
    bf16 = mybir.dt.bfloat16
    i16 = mybir.dt.int16
    eq = mybir.AluOpType.is_equal
    mult = mybir.AluOpType.mult
    AF = mybir.ActivationFunctionType

    nc = bacc.Bacc("TRN2", target_bir_lowering=False, debug=False,
                   num_devices=NCORES, num_swdge_queues=NQUEUES)
    featsB = nc.dram_tensor("featsB", [L * N, C], bf16, kind="ExternalInput")
    wmat = nc.dram_tensor("wmat", [C, C], bf16, kind="ExternalInput")
    iotaT = nc.dram_tensor("iotaT", [128, KSEL * W], bf16,
                           kind="ExternalInput")
    idx1 = nc.dram_tensor("idx1", [128, plan1.tot_slots // 16], i16,
                          kind="ExternalInput")
    idx2 = nc.dram_tensor("idx2", [128, plan2.tot_slots // 16], i16,
                          kind="ExternalInput")
    aux1 = nc.dram_tensor("aux1", [128, plan1.tot_aux], bf16,
                          kind="ExternalInput")
    aux2 = nc.dram_tensor("aux2", [128, plan2.tot_aux], bf16,
                          kind="ExternalInput")
    outp = nc.dram_tensor("outp", [L, SHARD, C], f32, kind="ExternalOutput")

    qn = [0]

    def q():
        qn[0] += 1
        return qn[0] % NQUEUES

    with tile.TileContext(nc) as tc:
        with (
            tc.tile_pool(name="const", bufs=1) as constp,
            tc.tile_pool(name="aux", bufs=3) as auxp,
            tc.tile_pool(name="idx", bufs=8) as idxp,
            tc.tile_pool(name="dst1", bufs=6) as dst1p,
            tc.tile_pool(name="dst2", bufs=6) as dst2p,
            tc.tile_pool(name="sel", bufs=6) as selp,
            tc.tile_pool(name="stg", bufs=4) as stgp,
            tc.tile_pool(name="zsb", bufs=2) as zsbp,
            tc.tile_pool(name="psU", bufs=2, space="PSUM") as psUp,
            tc.tile_pool(name="psZ", bufs=1, space="PSUM") as psZp,
            tc.tile_pool(name="psO", bufs=2, space="PSUM") as psOp,
            tc.tile_pool(name="dram", bufs=4, space="DRAM") as dramp,
        ):
            w_t = constp.tile([C, C], bf16, name="w_t")
            nc.sync.dma_start(w_t[:], wmat[:])
            io_t = constp.tile([128, KSEL * W], bf16, name="io_t")
            nc.sync.dma_start(io_t[:], iotaT[:])

            ztbs = []

            def emit_mat(plan, idxT, auxT, dstp, l, srcs, consume_block):
                """Gathers (chunk-interleaved lo/hi, balanced by block
                coverage) + per-block matmul consumption for one (l, mat)."""
                ao0 = int(plan.auxoff[l, 0, 0, 0])
                a_end = (int(plan.auxoff[l, 1, NBPC - 1, 1])
                         + 2 * int(plan.nt[l, 1, NBPC - 1, 1]))
                vt = auxp.tile([128, a_end - ao0], bf16, tag="aux")
                nc.sync.dma_start(vt[:], auxT[:, ao0:a_end])

                runs = []
                for h in range(2):
                    s0 = int(plan.segoff[l, h, 0, 0])
                    ln = (int(plan.segoff[l, h, NBPC - 1, 1])
                          + int(plan.slots[l, h, NBPC - 1, 1]) - s0)
                    chunks = []
                    o = 0
                    while o < ln:
                        n = min(CAP, ln - o)
                        chunks.append((s0 + o, n))
                        o += n
                    runs.append(chunks)

                tiles = {}          # (h, i) -> dst tile
                covered = [0, 0]    # absolute slot end gathered per half
                nextb = [0]

                def emit_ready():
                    while nextb[0] < NBPC:
                        b = nextb[0]
                        ends = [int(plan.segoff[l, h, b, 1])
                                + int(plan.slots[l, h, b, 1])
                                for h in range(2)]
                        if covered[0] < ends[0] or covered[1] < ends[1]:
                            return
                        # tiles of this block in (w, h, t) order so each
                        # window forms one contiguous accumulation chain
                        tl = []
                        for w in range(2):
                            for h in range(2):
                                seg0 = int(plan.segoff[l, h, b, w])
                                nt = int(plan.nt[l, h, b, w])
                                run0 = int(plan.segoff[l, h, 0, 0])
                                for t in range(nt):
                                    s = seg0 + t * 128 - run0
                                    ti, loc = s // CAP, (s % CAP) // 128
                                    tl.append((tiles[(h, ti)], loc,
                                               (h, b, w, t)))
                        consume_block(b, tl, vt, ao0)
                        nextb[0] += 1

                def blocks_done(h):
                    bc = 0
                    while bc < NBPC and (int(plan.segoff[l, h, bc, 1])
                                         + int(plan.slots[l, h, bc, 1])
                                         <= covered[h]):
                        bc += 1
                    return bc

                ci = [0, 0]
                while ci[0] < len(runs[0]) or ci[1] < len(runs[1]):
                    if ci[0] >= len(runs[0]):
                        h = 1
                    elif ci[1] >= len(runs[1]):
                        h = 0
                    else:
                        h = 0 if blocks_done(0) <= blocks_done(1) else 1
                    s0, n = runs[h][ci[h]]
                    dst = dstp.tile([128, CAP // 128, C], bf16, tag="dst")
                    it = idxp.tile([128, CAP // 16], i16, tag="idx")
                    nc.sync.dma_start(
                        it[:, :n // 16], idxT[:, s0 // 16:(s0 + n) // 16])
                    nc.gpsimd.dma_gather(
                        dst[:, :n // 128, :], srcs[h],
                        it[:, :n // 16], n, n, C,
                        single_packet=False, queue_num=q())
                    tiles[(h, ci[h])] = dst
                    covered[h] = s0 + n
                    ci[h] += 1
                    emit_ready()
                emit_ready()
                assert nextb[0] == NBPC

            def sel_groups(tl, vt, ao0, plan, l):
                """Yield (sel, k, tile, loc, w, first, last): sel columns
                [k*W:(k+1)*W] for consecutive tiles of one sub-segment."""
                nw = [0, 0]
                for _, _, (h, b, w, t) in tl:
                    nw[w] += 1
                done = [0, 0]
                i = 0
                while i < len(tl):
                    h, b, w, t0 = tl[i][2]
                    nt = int(plan.nt[l, h, b, w])
                    K = min(KSEL, nt - t0)
                    ao = int(plan.auxoff[l, h, b, w]) - ao0
                    sel = selp.tile([128, KSEL * W], bf16, tag="sel")
                    s3 = sel[:, :K * W].rearrange("p (k r) -> p k r", k=K)
                    rl_b = vt[:, ao + t0:ao + t0 + K].to_broadcast(
                        [128, K, W])
                    io_v = io_t[:, :K * W].rearrange(
                        "p (k r) -> p k r", k=K)
                    nc.vector.tensor_tensor(out=s3, in0=rl_b, in1=io_v,
                                            op=eq)
                    v_b = vt[:, ao + nt + t0:ao + nt + t0 + K
                             ].to_broadcast([128, K, W])
                    nc.vector.tensor_tensor(out=s3, in0=s3, in1=v_b,
                                            op=mult)
                    for k in range(K):
                        yield (sel, k, tl[i + k][0], tl[i + k][1], w,
                               done[w] == 0, done[w] == nw[w] - 1)
                        done[w] += 1
                    i += K

            def spmm1_scale(l):
                zshw = dramp.tile([128, NBPC * C], bf16, tag="zshw")
                ztbw = dramp.tile([TBL, C], bf16, tag="ztbw",
                                  addr_space="Shared")
                ztbs.append(ztbw)
                zsb = zsbp.tile([128, NBPC * C], bf16, tag="zsb")
                src_lo = featsB[l * N:l * N + HALF, :]
                src_hi = featsB[l * N + HALF:(l + 1) * N, :]

                def consume(b, tl, vt, ao0):
                    psU = psUp.tile([128, 128], f32)
                    for sel, k, dt, loc, w, first, last in sel_groups(
                            tl, vt, ao0, plan1, l):
                        nc.tensor.matmul(
                            psU[:, w * W:(w + 1) * W], lhsT=dt[:, loc, :],
                            rhs=sel[:, k * W:(k + 1) * W],
                            start=first, stop=last)
                    ut = stgp.tile([128, 128], bf16, tag="ut")
                    nc.scalar.activation(ut[:], psU[:], AF.Copy)
                    psZ = psZp.tile([128, 128], f32)
                    nc.tensor.matmul(psZ[:], lhsT=ut[:], rhs=w_t[:],
                                     start=True, stop=True)
                    nc.scalar.activation(zsb[:, b * C:(b + 1) * C], psZ[:],
                                         AF.Copy)

                emit_mat(plan1, idx1, aux1, dst1p, l, (src_lo, src_hi),
                         consume)
                nc.scalar.dma_start(zshw[:], zsb[:])
                nc.gpsimd.collective_compute(
                    "AllGather", mybir.AluOpType.bypass,
                    replica_groups=[list(range(NCORES))],
                    ins=[zshw.opt()], outs=[ztbw.opt()])

            def spmm2_scale(l):
                ztbw = ztbs[l]
                src_lo = ztbw[:HALF, :]
                src_hi = ztbw[HALF:, :]

                def consume(b, tl, vt, ao0):
                    psO = psOp.tile([128, 128], f32)
                    for sel, k, dt, loc, w, first, last in sel_groups(
                            tl, vt, ao0, plan2, l):
                        nc.tensor.matmul(
                            psO[w * W:(w + 1) * W, :],
                            lhsT=sel[:, k * W:(k + 1) * W],
                            rhs=dt[:, loc, :],
                            start=first, stop=last)
                    ot = stgp.tile([128, 128], f32, tag="ot")
                    nc.vector.tensor_copy(ot[:], psO[:])
                    nc.scalar.dma_start(outp[l, b * BLK:(b + 1) * BLK, :],
                                        ot[:])

                emit_mat(plan2, idx2, aux2, dst2p, l, (src_lo, src_hi),
                         consume)

            done1 = done2 = 0
            while done2 < scales:
                if done1 < scales:
                    spmm1_scale(done1)
                    done1 += 1
                if done1 >= min(2, scales) and done2 < done1:
                    spmm2_scale(done2)
                    done2 += 1
    nc.compile()
    return nc


def kernel(**inputs):
    _install_hook_stub()
    from concourse.bass_utils import run_bass_kernel_spmd

    feats = np.asarray(inputs["features"], np.float32)        # [N, L, C]
    featsB = np.ascontiguousarray(
        feats.transpose(1, 0, 2).reshape(L * N, C)).astype(BF16)
    wmat = np.asarray(inputs["weight_matrix"], np.float32).astype(BF16)

    plan1, idx1, aux1, plan2, idx2, aux2 = _preprocess(
        np.asarray(inputs["phi_indices"]), np.asarray(inputs["phi_values"]),
        np.asarray(inputs["phi_inverse_indices"]),
        np.asarray(inputs["phi_inverse_values"]),
        np.asarray(inputs["diagonal_weight_filter"]))

    scales = int(os.environ.get("DGW_SCALES", L))
    nc = _build(plan1, plan2, scales)

    iotaT = np.ascontiguousarray(np.tile(
        np.arange(W, dtype=np.float32)[None, :], (128, KSEL))).astype(BF16)

    in_maps = []
    for c in range(NCORES):
        in_maps.append(dict(
            featsB=featsB, wmat=wmat, iotaT=iotaT,
            idx1=np.ascontiguousarray(idx1[c]),
            idx2=np.ascontiguousarray(idx2[c]),
            aux1=np.ascontiguousarray(aux1[c]),
            aux2=np.ascontiguousarray(aux2[c])))
    res = run_bass_kernel_spmd(nc, in_maps, core_ids=list(range(NCORES)))
    kernel.last_results = res

    shards = np.stack([res.results[c]["outp"] for c in range(NCORES)])
    blocks = shards.reshape(NCORES, L, NBPC, BLK, C).transpose(2, 0, 3, 1, 4)
    out = blocks.reshape(NBPC * NCORES * BLK, L, C)[:N]
    return np.ascontiguousarray(out)
